# revision 5
# baseline (speedup 1.0000x reference)
"""BiLSTM-CRF NLL kernel for Trainium2 (8 NeuronCores, data-parallel over batch).

Full inputs in, full (scalar) output out.  Internally:
  - batch B=64 sharded 8 ways (8 sequences per core)
  - per core: embedding gather (indirect DMA) -> transpose -> x-gate precompute
    (bf16 matmuls) -> 512-step fwd+bwd LSTM recurrence (hidden-on-partitions,
    tanh via scaled sigmoid) -> fc emissions (interleaved into the LSTM loop
    as both directions' hidden states become available) -> exp-domain CRF
    split into a forward alpha chain and a backward beta chain that meet in
    the middle (256 sequential steps instead of 511) -> per-core partials
  - host: gold-path start/end/transition score (pure index arithmetic on
    inputs) + final combine of per-core partials.
"""

import ml_dtypes
import numpy as np

import concourse.bass as bass
import concourse.mybir as mybir
import concourse.tile as tile
from concourse import bacc
from concourse.bass_utils import run_bass_kernel_spmd
from concourse.masks import make_identity

F32 = mybir.dt.float32
BF16 = mybir.dt.bfloat16
I32 = mybir.dt.int32
AF = mybir.ActivationFunctionType
OP = mybir.AluOpType

V, E, H, K = 32000, 128, 128, 9       # vocab, emb dim, per-dir hidden, tags
G4 = 4 * H                            # 512: packed gate width
B, T = 64, 512
NCORES = 8
BL = B // NCORES                      # 8 sequences per core
N = T * BL                            # 4096 tokens per core
NCH = N // 128                        # 32 gather chunks of 128 tokens
NEMB = N // 512                       # 8 chunks of 512 tokens (matmul free dim)
CRF_SHIFT = float(np.log(K))          # exp-domain per-step shift
RENORM = 16
TM = T // 2                           # CRF meet point: alpha reaches t=TM-1

_CACHE = {}


def _build_program():
    nc = bacc.Bacc(None, target_bir_lowering=False)

    # ---- DRAM parameters (per-core values supplied via in_maps) ----
    emb_h = nc.declare_dram_parameter("emb", [V, E], F32, isOutput=False)
    tok_h = nc.declare_dram_parameter("tok", [128, NCH], I32, isOutput=False)
    y1h_h = nc.declare_dram_parameter("y1h", [K, N], F32, isOutput=False)
    wih_h = nc.declare_dram_parameter("wih", [2, E, G4], BF16, isOutput=False)
    whh_h = nc.declare_dram_parameter("whh", [2, H, G4], BF16, isOutput=False)
    bias_h = nc.declare_dram_parameter("bias", [2, H, 4], F32, isOutput=False)
    fcw_h = nc.declare_dram_parameter("fcw", [2, H, K], BF16, isOutput=False)
    fcb_h = nc.declare_dram_parameter("fcb", [K, 1], F32, isOutput=False)
    trans_h = nc.declare_dram_parameter("trans", [K, K], F32, isOutput=False)
    transT_h = nc.declare_dram_parameter("transT", [K, K], F32, isOutput=False)
    start_h = nc.declare_dram_parameter("startv", [K, 1], F32, isOutput=False)
    end_h = nc.declare_dram_parameter("endv", [K, 1], F32, isOutput=False)
    out_h = nc.declare_dram_parameter("out", [1, 8], F32, isOutput=True)

    with tile.TileContext(nc) as tc:
        with (
            tc.tile_pool(name="const", bufs=1) as cpool,
            tc.tile_pool(name="big", bufs=1) as bpool,
            tc.tile_pool(name="work", bufs=2) as wpool,
            tc.tile_pool(name="psA", bufs=2, space="PSUM") as psA,
            tc.tile_pool(name="psB", bufs=2, space="PSUM") as psB,
            tc.tile_pool(name="psC", bufs=2, space="PSUM") as psC,
        ):
            # ---------------- constants / weights to SBUF ----------------
            ident = cpool.tile([128, 128], F32, tag="ident")
            make_identity(nc, ident[:])
            identD = cpool.tile([128, 128], F32, tag="identD")
            nc.vector.tensor_copy(out=identD[:], in_=ident[:])
            ident_bf = cpool.tile([128, 128], BF16, tag="ident_bf")
            nc.vector.tensor_copy(out=ident_bf[:], in_=ident[:])

            tok = cpool.tile([128, NCH], I32, tag="tok")
            nc.sync.dma_start(out=tok[:], in_=tok_h[:, :])

            wih = cpool.tile([128, 2, G4], BF16, tag="wih")
            nc.sync.dma_start(out=wih[:], in_=wih_h.rearrange("d e g -> e d g"))
            whh = cpool.tile([128, 2, G4], BF16, tag="whh")
            nc.sync.dma_start(out=whh[:], in_=whh_h.rearrange("d e g -> e d g"))
            biases = cpool.tile([128, 2, 4], F32, tag="biases")
            nc.sync.dma_start(out=biases[:], in_=bias_h.rearrange("d e g -> e d g"))
            fcw = cpool.tile([128, 2, K], BF16, tag="fcw")
            nc.sync.dma_start(out=fcw[:], in_=fcw_h.rearrange("d e g -> e d g"))
            fcb = cpool.tile([K, 1], F32, tag="fcb")
            nc.sync.dma_start(out=fcb[:], in_=fcb_h[:, :])
            trans = cpool.tile([K, K], F32, tag="trans")
            nc.sync.dma_start(out=trans[:], in_=trans_h[:, :])
            transT = cpool.tile([K, K], F32, tag="transT")
            nc.sync.dma_start(out=transT[:], in_=transT_h[:, :])
            startv = cpool.tile([K, 1], F32, tag="startv")
            nc.sync.dma_start(out=startv[:], in_=start_h[:, :])
            endv = cpool.tile([K, 1], F32, tag="endv")
            nc.sync.dma_start(out=endv[:], in_=end_h[:, :])

            ones9 = cpool.tile([K, 1], F32, tag="ones9")
            nc.vector.memset(ones9[:], 1.0)
            ones1x9 = cpool.tile([1, K], F32, tag="ones1x9")
            nc.vector.memset(ones1x9[:], 1.0)
            ones9xb = cpool.tile([K, BL], F32, tag="ones9xb")
            nc.vector.memset(ones9xb[:], 1.0)

            # exp-domain CRF tables (exp/ln table set, loaded before sigmoid set)
            shiftc = cpool.tile([K, 1], F32, tag="shiftc")
            nc.vector.memset(shiftc[:], -CRF_SHIFT)
            transE = cpool.tile([K, K], F32, tag="transE")
            nc.scalar.activation(transE[:], trans[:], AF.Exp, bias=shiftc[:])
            transET = cpool.tile([K, K], F32, tag="transET")
            nc.scalar.activation(transET[:], transT[:], AF.Exp, bias=shiftc[:])
            estart = cpool.tile([K, 1], F32, tag="estart")
            nc.scalar.activation(estart[:], startv[:], AF.Exp)
            eend = cpool.tile([K, 1], F32, tag="eend")
            nc.scalar.activation(eend[:], endv[:], AF.Exp)

            # ---------------- phase 1: gather + transpose + x-gates ------
            # token-major gather chunks: token n=c*128+p on partition p
            xsT = bpool.tile([128, N], BF16, tag="xst")  # (E, tokens)
            for c in range(NCH):
                gch = wpool.tile([128, E], F32, tag="gchunk", bufs=NCH, name=f"gch{c}")
                nc.gpsimd.indirect_dma_start(
                    out=gch[:],
                    out_offset=None,
                    in_=emb_h[:, :],
                    in_offset=bass.IndirectOffsetOnAxis(ap=tok[:, c : c + 1], axis=0),
                )
                pt = psA.tile([128, 128], F32, tag="pA")
                nc.tensor.transpose(out=pt[:], in_=gch[:], identity=identD[:])
                nc.vector.tensor_copy(out=xsT[:, c * 128 : (c + 1) * 128], in_=pt[:])

            # xg layout: [t(512)][dir(2)][gate(4)][b(8)] along free dim
            xg = bpool.tile([128, T, 2, 32], BF16, tag="xg")
            for d in range(2):
                for g in range(4):
                    for c in range(NEMB):
                        px = psB.tile([128, 512], F32, tag="pB")
                        nc.tensor.matmul(
                            out=px[:],
                            lhsT=wih[:, d, g * 128 : (g + 1) * 128],
                            rhs=xsT[:, c * 512 : (c + 1) * 512],
                            start=True,
                            stop=True,
                        )
                        # dest: 64 timesteps x 8 batch, strided into xg
                        dst = xg[:, c * 64 : (c + 1) * 64, d, g * 8 : (g + 1) * 8]
                        nc.scalar.activation(
                            dst,
                            px[:].rearrange("p (t b) -> p t b", b=8),
                            AF.Identity,
                            bias=biases[:, d, g : g + 1],
                        )

            # ---------------- phase 2: LSTM recurrence -------------------
            hs = bpool.tile([128, 2, T, BL], BF16, tag="hs")  # stores h/2
            h0 = cpool.tile([128, BL], BF16, tag="h0")
            nc.vector.memset(h0[:], 0.0)
            cst = [cpool.tile([128, BL], F32, tag=f"c{d}", name=f"cst{d}") for d in range(2)]
            for d in range(2):
                nc.vector.memset(cst[d][:], 0.0)

            # phase 3 state, written incrementally as fc chunks complete
            Ep = bpool.tile([K, T, BL], F32, tag="g_ep", name="Ep")
            emit_acc = cpool.tile([K, NEMB], F32, tag="emit_acc")

            def emit_fc_chunk(c):
                pe = psB.tile([K, 512], F32, tag="pB")
                nc.tensor.matmul(
                    out=pe[:],
                    lhsT=fcw[:, 0, :],
                    rhs=hs[:, 0, c * 64 : (c + 1) * 64, :].rearrange(
                        "p t b -> p (t b)"
                    ),
                    start=True,
                    stop=False,
                )
                nc.tensor.matmul(
                    out=pe[:],
                    lhsT=fcw[:, 1, :],
                    rhs=hs[:, 1, c * 64 : (c + 1) * 64, :].rearrange(
                        "p t b -> p (t b)"
                    ),
                    start=False,
                    stop=True,
                )
                emc = wpool.tile([K, 512], F32, tag="emc")
                nc.scalar.activation(emc[:], pe[:], AF.Identity, bias=fcb[:])
                nc.scalar.activation(
                    Ep[:, c * 64 : (c + 1) * 64, :].rearrange("p t b -> p (t b)"),
                    emc[:],
                    AF.Exp,
                )
                y1c = wpool.tile([K, 512], F32, tag="y1c")
                nc.sync.dma_start(out=y1c[:], in_=y1h_h[:, c * 512 : (c + 1) * 512])
                dume = wpool.tile([K, 512], F32, tag="dume", bufs=1)
                nc.vector.scalar_tensor_tensor(
                    out=dume[:],
                    in0=emc[:],
                    scalar=0.0,
                    in1=y1c[:],
                    op0=OP.add,
                    op1=OP.mult,
                    accum_out=emit_acc[:, c : c + 1],
                )

            def emit_mms(s, d):
                t = s if d == 0 else T - 1 - s
                tprev = (s - 1) if d == 0 else (T - s)
                ps = psC.tile([128, 32], F32, tag="pstep", name=f"ps{d}_{s}")
                nc.tensor.matmul(
                    out=ps[:], lhsT=ident_bf[:], rhs=xg[:, t, d, :],
                    start=True, stop=False,
                )
                hprev = h0[:] if s == 0 else hs[:, d, tprev, :]
                for g in range(4):
                    nc.tensor.matmul(
                        out=ps[:, g * 8 : (g + 1) * 8],
                        lhsT=whh[:, d, g * 128 : (g + 1) * 128],
                        rhs=hprev, start=False, stop=(g == 3),
                    )
                return ps

            def emit_sig1(s, d, ps):
                sg = wpool.tile([128, 32], F32, tag=f"sg{d}", name=f"sg{d}_{s}")
                nc.scalar.activation(sg[:], ps[:], AF.Sigmoid)
                return sg

            def emit_tvc(s, d, sg):
                tt = wpool.tile([128, BL], F32, tag=f"tt{d}", name=f"tt{d}_{s}")
                nc.vector.scalar_tensor_tensor(
                    out=tt[:], in0=sg[:, 24:32], scalar=0.5, in1=sg[:, 0:8],
                    op0=OP.subtract, op1=OP.mult,
                )
                vv = wpool.tile([128, BL], F32, tag=f"vv{d}", name=f"vv{d}_{s}")
                nc.vector.tensor_tensor(
                    out=vv[:], in0=sg[:, 8:16], in1=cst[d][:], op=OP.mult
                )
                nc.vector.scalar_tensor_tensor(
                    out=cst[d][:], in0=tt[:], scalar=2.0, in1=vv[:],
                    op0=OP.mult, op1=OP.add,
                )

            def emit_sig2(s, d):
                sc = wpool.tile([128, BL], F32, tag=f"sc{d}", name=f"sc{d}_{s}")
                nc.scalar.activation(sc[:], cst[d][:], AF.Sigmoid, scale=2.0)
                return sc

            def emit_h(s, d, sg, sc):
                t = s if d == 0 else T - 1 - s
                nc.vector.scalar_tensor_tensor(
                    out=hs[:, d, t, :], in0=sc[:], scalar=0.5, in1=sg[:, 16:24],
                    op0=OP.subtract, op1=OP.mult,
                )

            # software-pipelined interleave: bwd runs half a step behind fwd.
            # DVE order keeps both chains' cell updates ahead of the emit ops
            # so neither chain's tvc stalls behind the other's sig2 wait.
            for s in range(T):
                ps_f = emit_mms(s, 0)
                sg_f = emit_sig1(s, 0, ps_f)
                ps_b = emit_mms(s, 1)
                sg_b = emit_sig1(s, 1, ps_b)
                emit_tvc(s, 0, sg_f)
                emit_tvc(s, 1, sg_b)
                sc_f = emit_sig2(s, 0)
                emit_h(s, 0, sg_f, sc_f)
                sc_b = emit_sig2(s, 1)
                emit_h(s, 1, sg_b, sc_b)
            # alpha needs chunk 0 and beta chunk 7 before the CRF starts;
            # the rest are injected into the CRF loop's engine slack, each
            # well before its chain reaches it (chunk c by iteration 64c).
            emit_fc_chunk(0)
            emit_fc_chunk(7)

            # ---------------- phase 4: CRF fwd/bwd meet-in-middle --------
            # alpha: a_t = e_t * (transE^T a_{t-1}),  t = 1 .. TM-1
            # beta:  b_t = transE (e_{t+1} * b_{t+1}), t = T-2 .. TM-1
            # logZ  = log(sum_i a_{TM-1}[i] b_{TM-1}[i]) + offs
            offs = cpool.tile([1, BL], F32, tag="offs")
            nc.vector.memset(offs[:], 0.0)
            Pa = cpool.tile([K, BL], F32, tag="Pa")
            Pb = cpool.tile([K, BL], F32, tag="Pb")
            nc.vector.tensor_scalar_mul(Pa[:], Ep[:, 0, :], estart[:])
            Qa = cpool.tile([K, BL], F32, tag="Qa")
            Qb = cpool.tile([K, BL], F32, tag="Qb")
            nc.vector.tensor_scalar_mul(Qa[:], ones9xb[:], eend[:])

            def emit_renorm(state_sb, which):
                # state_sb: SBUF [K, BL]; renormalize in place, log into offs
                psum_s = psC.tile([1, BL], F32, tag="psmall", name=f"ren{which}")
                nc.tensor.matmul(
                    out=psum_s[:], lhsT=ones9[:], rhs=state_sb[:], start=True,
                    stop=True,
                )
                rec = wpool.tile([1, BL], F32, tag=f"rec{which}")
                nc.vector.reciprocal(rec[:], psum_s[:])
                psum_b = psC.tile([K, BL], F32, tag="pstep", name=f"renb{which}")
                nc.tensor.matmul(
                    out=psum_b[:], lhsT=ones1x9[:], rhs=rec[:], start=True,
                    stop=True,
                )
                nc.vector.tensor_tensor(
                    out=state_sb[:], in0=state_sb[:], in1=psum_b[:], op=OP.mult
                )
                # off-chain: offs += ln(sum)
                lgs = wpool.tile([1, BL], F32, tag=f"lgs{which}")
                nc.scalar.activation(lgs[:], psum_s[:], AF.Ln)
                nc.vector.tensor_tensor(
                    out=offs[:], in0=offs[:], in1=lgs[:], op=OP.add
                )

            curA, nxtA = Pa, Pb
            curB, nxtB = Qa, Qb
            # iteration r: alpha consumes e_r, beta consumes e_{T-r}
            for r in range(1, TM):
                ppA = psC.tile([K, BL], F32, tag="pstep", name=f"ppA{r}")
                nc.tensor.matmul(
                    out=ppA[:], lhsT=transE[:], rhs=curA[:], start=True, stop=True
                )
                nc.vector.tensor_tensor(
                    out=nxtA[:], in0=ppA[:], in1=Ep[:, r, :], op=OP.mult
                )
                curA, nxtA = nxtA, curA
                # beta half-step: w = e_{T-r} * curB ; curB' = transE @ w
                wB = wpool.tile([K, BL], F32, tag="wB", name=f"wB{r}")
                nc.vector.tensor_tensor(
                    out=wB[:], in0=curB[:], in1=Ep[:, T - r, :], op=OP.mult
                )
                ppB = psC.tile([K, BL], F32, tag="pstep", name=f"ppB{r}")
                nc.tensor.matmul(
                    out=ppB[:], lhsT=transET[:], rhs=wB[:], start=True, stop=True
                )
                nc.vector.tensor_copy(out=nxtB[:], in_=ppB[:])
                curB, nxtB = nxtB, curB
                if r % RENORM == 0:
                    emit_renorm(curA, f"A{r}")
                    emit_renorm(curB, f"B{r}")
                fc_late = {4: 1, 12: 6, 36: 2, 44: 5, 72: 3, 80: 4}
                if r in fc_late:
                    emit_fc_chunk(fc_late[r])
            # final beta half-step consuming e_TM, then combine at t=TM-1
            wB = wpool.tile([K, BL], F32, tag="wB", name="wBfin")
            nc.vector.tensor_tensor(
                out=wB[:], in0=curB[:], in1=Ep[:, TM, :], op=OP.mult
            )
            ppB = psC.tile([K, BL], F32, tag="pstep", name="ppBfin")
            nc.tensor.matmul(
                out=ppB[:], lhsT=transET[:], rhs=wB[:], start=True, stop=True
            )
            nc.vector.tensor_copy(out=nxtB[:], in_=ppB[:])
            curB = nxtB

            # logZ = ln(sum_i a[i]*b[i]) + offs, summed over b
            pz = wpool.tile([K, BL], F32, tag="pz")
            nc.vector.tensor_tensor(out=pz[:], in0=curA[:], in1=curB[:],
                                    op=OP.mult)
            psum_z = psC.tile([1, BL], F32, tag="psmall", name="pzsum")
            nc.tensor.matmul(
                out=psum_z[:], lhsT=ones9[:], rhs=pz[:], start=True, stop=True
            )
            lz = wpool.tile([1, BL], F32, tag="lz")
            nc.scalar.activation(lz[:], psum_z[:], AF.Ln)
            nc.vector.tensor_tensor(out=lz[:], in0=lz[:], in1=offs[:], op=OP.add)

            out_sb = cpool.tile([1, 8], F32, tag="out_sb")
            nc.vector.memset(out_sb[:], 0.0)
            nc.vector.tensor_reduce(
                out=out_sb[:, 0:1], in_=lz[:], axis=mybir.AxisListType.X, op=OP.add
            )
            # emit total: reduce chunks then partitions (via ones matmul)
            em9 = wpool.tile([K, 1], F32, tag="em9")
            nc.vector.tensor_reduce(
                out=em9[:], in_=emit_acc[:], axis=mybir.AxisListType.X, op=OP.add
            )
            psum_e = psC.tile([1, 1], F32, tag="psmall", name="pesum")
            nc.tensor.matmul(
                out=psum_e[:], lhsT=ones9[:], rhs=em9[:], start=True, stop=True
            )
            nc.vector.tensor_copy(out=out_sb[:, 1:2], in_=psum_e[:])
            nc.sync.dma_start(out=out_h[:, :], in_=out_sb[:])

    nc.finalize()
    return nc


def _prep_core_inputs(ci, emb, wih_T, whh_T, bias_np, fcw_T, fcb, trans, transT,
                      startv, endv, x, y1h_full):
    xl = x[ci * BL : (ci + 1) * BL]                     # (8, 512)
    flat = xl.T.reshape(-1)                             # token order n = t*8+b
    tok = np.ascontiguousarray(flat.reshape(NCH, 128).T.astype(np.int32))
    y1h = y1h_full[:, ci * N : (ci + 1) * N]
    return {
        "emb": emb,
        "tok": tok,
        "y1h": np.ascontiguousarray(y1h),
        "wih": wih_T,
        "whh": whh_T,
        "bias": bias_np,
        "fcw": fcw_T,
        "fcb": fcb,
        "trans": trans,
        "transT": transT,
        "startv": startv,
        "endv": endv,
    }


def _host_prep(inputs):
    f32 = np.float32
    bf16 = ml_dtypes.bfloat16
    emb = np.ascontiguousarray(np.asarray(inputs["emb"], dtype=f32))
    x = np.asarray(inputs["x"]).astype(np.int64)
    y = np.asarray(inputs["y"]).astype(np.int64)
    perm = [0, 1, 3, 2]  # pytorch [i,f,g,o] -> kernel [i,f,o,g]
    gate_scale_x = np.array([1.0, 1.0, 1.0, 2.0], dtype=f32)
    gate_scale_h = np.array([2.0, 2.0, 2.0, 4.0], dtype=f32)

    def prep_w(w, scales):
        # w: (4H, E) -> transposed (E, 4H), gate-reordered + scaled
        wt = np.asarray(w, dtype=f32).T.reshape(-1, 4, H)[:, perm, :]
        wt = wt * scales[None, :, None]
        return np.ascontiguousarray(wt.reshape(-1, G4).astype(bf16))

    wih_T = np.stack(
        [prep_w(inputs["w_ih_f"], gate_scale_x), prep_w(inputs["w_ih_b"], gate_scale_x)]
    )
    whh_T = np.stack(
        [prep_w(inputs["w_hh_f"], gate_scale_h), prep_w(inputs["w_hh_b"], gate_scale_h)]
    )

    def prep_b(bi, bh):
        bb = (np.asarray(bi, dtype=f32) + np.asarray(bh, dtype=f32)).reshape(4, H)
        bb = bb[perm] * gate_scale_x[:, None]
        return np.ascontiguousarray(bb.T)  # (H, 4)

    bias_np = np.stack(
        [
            prep_b(inputs["b_ih_f"], inputs["b_hh_f"]),
            prep_b(inputs["b_ih_b"], inputs["b_hh_b"]),
        ]
    )
    fcw = np.asarray(inputs["fc_w"], dtype=f32)         # (K, 2H)
    fcw_T = np.stack(
        [
            np.ascontiguousarray((2.0 * fcw[:, :H].T).astype(bf16)),  # (H, K)
            np.ascontiguousarray((2.0 * fcw[:, H:].T).astype(bf16)),
        ]
    )
    fcb = np.ascontiguousarray(np.asarray(inputs["fc_b"], dtype=f32).reshape(K, 1))
    trans = np.ascontiguousarray(np.asarray(inputs["trans"], dtype=f32))
    transT = np.ascontiguousarray(trans.T)
    startv = np.ascontiguousarray(
        np.asarray(inputs["start_t"], dtype=f32).reshape(K, 1)
    )
    endv = np.ascontiguousarray(np.asarray(inputs["end_t"], dtype=f32).reshape(K, 1))

    # one-hot of y in (k, n) layout, n = t*BL + b within each core's shard
    y1h_full = np.zeros((K, B * T), dtype=f32)
    for ci in range(NCORES):
        yl = y[ci * BL : (ci + 1) * BL]                 # (8, 512)
        yflat = yl.T.reshape(-1)
        y1h_full[yflat, ci * N + np.arange(N)] = 1.0

    # gold-path score pieces that depend only on (y, small params)
    st = np.asarray(inputs["start_t"], dtype=np.float64)
    en = np.asarray(inputs["end_t"], dtype=np.float64)
    tr = np.asarray(inputs["trans"], dtype=np.float64)
    gold_const = (
        st[y[:, 0]].sum() + tr[y[:, :-1], y[:, 1:]].sum() + en[y[:, -1]].sum()
    )
    return (emb, wih_T, whh_T, bias_np, fcw_T, fcb, trans, transT, startv, endv,
            x, y1h_full, gold_const)


def _get_nc():
    if "nc" not in _CACHE:
        _CACHE["nc"] = _build_program()
    return _CACHE["nc"]


def run_kernel(inputs, trace=False):
    (emb, wih_T, whh_T, bias_np, fcw_T, fcb, trans, transT, startv, endv, x,
     y1h_full, gold_const) = _host_prep(inputs)
    in_maps = [
        _prep_core_inputs(ci, emb, wih_T, whh_T, bias_np, fcw_T, fcb, trans,
                          transT, startv, endv, x, y1h_full)
        for ci in range(NCORES)
    ]
    nc = _get_nc()
    res = run_bass_kernel_spmd(nc, in_maps, list(range(NCORES)), trace=trace)
    total = 0.0
    for r in res.results:
        o = np.asarray(r["out"], dtype=np.float64).reshape(-1)
        total += o[0] - o[1]
    # transE carries a -log(K) shift per CRF step; restore the constant
    nll = total + B * (T - 1) * CRF_SHIFT - gold_const
    return np.float32(nll), res


def kernel(**inputs) -> np.ndarray:
    val, _ = run_kernel(inputs, trace=False)
    return np.float32(val)
